# revision 8
# baseline (speedup 1.0000x reference)
"""Trainium2 Bass kernel for a single-head causal attention block (v6).

v6: the projection phase is fully folded into the x^T DMA stream:
  - each arriving d-tile feeds FOUR N=1024 bf16 matmuls (QK for s-blocks
    0|1 and 2|3, col-packed V^T for 0|1 and 2|3) so the PE is dense from
    the first tile and all projections finish with the DMA
  - one universal 4-slot x 2-bank PSUM pool serves accumulators, score
    tiles, PV accumulator and fin tiles
  - attention then runs as one continuous ACT-paced exp stream
"""

import numpy as np
import ml_dtypes

import concourse.bass as bass
import concourse.bacc as bacc
import concourse.mybir as mybir
from concourse.tile import TileContext
from concourse.bass_utils import run_bass_kernel_spmd

B, S, D, H = 8, 2048, 1024, 64
NCORES = 8
PT = 128           # partition tile
SB = 512           # s-block width
NSB = S // SB      # 4
NDT = D // PT      # 8 d-tiles
TPB = SB // PT     # 4 t-tiles per s-block
HS = S // 2        # 1024: merged two-s-block moving width
BF16 = mybir.dt.bfloat16
F32 = mybir.dt.float32
BF16NP = ml_dtypes.bfloat16

_NC_CACHE = {}


def _build():
    nc = bacc.Bacc()
    xt_d = nc.dram_tensor("xt", [D, S], BF16, kind="ExternalInput")
    cst16_d = nc.dram_tensor("cst16", [PT, 1728], BF16, kind="ExternalInput")
    cst32_d = nc.dram_tensor("cst32", [PT, H + 3], F32, kind="ExternalInput")
    y_d = nc.dram_tensor("y", [S, H], F32, kind="ExternalOutput")

    scale = 1.0 / float(np.sqrt(H))

    with TileContext(nc) as tc:
        with (
            tc.tile_pool(name="consts", bufs=1) as consts,
            tc.tile_pool(name="persist", bufs=1) as persist,
            tc.tile_pool(name="work", bufs=2) as work,
            tc.tile_pool(name="expp", bufs=3) as expp,
            tc.tile_pool(name="otp", bufs=2) as otp,
            tc.tile_pool(name="yp", bufs=2) as yp,
            tc.tile_pool(name="small", bufs=4) as small,
            tc.tile_pool(name="ups", bufs=4, space="PSUM") as ups,
        ):
            # ---- x^T first (critical path), then two packed const DMAs
            xt_sb = persist.tile([PT, NDT, S], BF16, tag="xt")
            for k in range(NDT):
                nc.sync.dma_start(out=xt_sb[:, k, :],
                                  in_=xt_d[k * PT:(k + 1) * PT, :])
            cst16 = consts.tile([PT, 1728], BF16, tag="cst16")
            nc.sync.dma_start(out=cst16, in_=cst16_d[:])
            cst32 = consts.tile([PT, H + 3], F32, tag="cst32")
            nc.sync.dma_start(out=cst32, in_=cst32_d[:])
            wqk_sb = cst16[:, 0:NDT * PT].rearrange("p (k m) -> p k m", k=NDT)
            wv_sb = cst16[:, NDT * PT:NDT * PT + NDT * H].rearrange(
                "p (k m) -> p k m", k=NDT)
            e12_sb = cst16[:, 1536:1536 + PT]
            idn2_sb = cst16[:, 1664:1664 + H]
            wo_sb = cst32[0:H + 1, 0:H + 1]
            bqk_sb = cst32[:, H + 1:H + 2]
            bv2_sb = cst32[:, H + 2:H + 3]

            # warm the ACT exp table while DMA streams in
            warm_in = small.tile([PT, 1], F32, tag="warm_in")
            nc.vector.memset(warm_in, 0.0)
            warm_out = small.tile([PT, 1], F32, tag="warm_out")
            nc.scalar.activation(warm_out, warm_in, mybir.ActivationFunctionType.Exp)

            qkt = [None] * NSB   # [128, SB] bf16: rows 0:64 Q^T, 64:128 K^T
            ktq = [None] * NSB   # [128, SB] bf16: rows 0:64 K^T, 64:128 Q^T
            vext = [None] * (NSB * TPB)  # [128, H+1] bf16: [V | 1]

            # ---- projection d-loop: 6 matmuls per arriving d-tile (PSUM bank
            # limit forces N=512): QK for all four s-blocks + V^T for 0|1.
            # V^T for 2|3 follows as a short dense pass on resident x^T.
            a01 = ups.tile([PT, HS], F32, tag="u")   # [Q^T;K^T] s-blocks 0|1
            a23 = ups.tile([PT, HS], F32, tag="u")   # [Q^T;K^T] s-blocks 2|3
            vv = ups.tile([PT, HS], F32, tag="u")    # V^T: rows 0:64 blocks 0|1,
            for k in range(NDT):                     #      rows 64:128 blocks 2|3
                for half, a in ((0, a01), (1, a23)):
                    for q in range(2):
                        nc.tensor.matmul(
                            a[:, q * SB:(q + 1) * SB], lhsT=wqk_sb[:, k, :],
                            rhs=xt_sb[:, k, half * HS + q * SB:half * HS + (q + 1) * SB],
                            start=(k == 0), stop=(k == NDT - 1),
                            skip_group_check=True)
                for q in range(2):
                    nc.tensor.matmul(
                        vv[0:H, q * SB:(q + 1) * SB], lhsT=wv_sb[:, k, :],
                        rhs=xt_sb[:, k, q * SB:(q + 1) * SB],
                        start=(k == 0), stop=(k == NDT - 1),
                        skip_group_check=True)
            for k in range(NDT):
                for q in range(2):
                    nc.tensor.matmul(
                        vv[H:PT, q * SB:(q + 1) * SB], lhsT=wv_sb[:, k, :],
                        rhs=xt_sb[:, k, HS + q * SB:HS + (q + 1) * SB],
                        start=(k == 0), stop=(k == NDT - 1),
                        skip_group_check=True)

            def finish_qk(j):
                a = a01 if j < 2 else a23
                qkt_j = persist.tile([PT, SB], BF16, tag=f"qkt{j}")
                nc.vector.tensor_scalar_add(qkt_j, a[:, (j % 2) * SB:(j % 2 + 1) * SB],
                                            bqk_sb)
                qkt[j] = qkt_j
                ps_sw = ups.tile([PT, SB], F32, tag="u")
                nc.tensor.matmul(ps_sw, lhsT=e12_sb, rhs=qkt_j, start=True, stop=True)
                ktq_j = persist.tile([PT, SB], BF16, tag=f"ktq{j}")
                nc.vector.tensor_copy(ktq_j, ps_sw)
                ktq[j] = ktq_j

            # V^T -> SBUF once (both packed halves, one op), then transposes
            vt2 = work.tile([PT, HS], BF16, tag="vt")

            def finish_v(jpair):
                # jpair 0 -> s-blocks 0|1 from rows 0:64; 1 -> 2|3 from 64:128
                base = 0 if jpair == 0 else H
                for jj in (2 * jpair, 2 * jpair + 1):
                    for u in range(TPB):
                        tt = jj * TPB + u
                        col = (jj % 2) * SB + u * PT
                        ps_vt = ups.tile([PT, H], BF16, tag="u")
                        nc.tensor.transpose(ps_vt, vt2[base:base + H, col:col + PT],
                                            idn2_sb[base:base + H, :])
                        vx = persist.tile([PT, H + 1], BF16, tag=f"vext{tt}")
                        nc.vector.tensor_copy(vx[:, 0:H], ps_vt)
                        nc.vector.memset(vx[:, H:H + 1], 1.0)
                        vext[tt] = vx

            def attn(j):
                ps_o = ups.tile([H + 1, SB], F32, tag="u")
                last_tt = (j + 1) * TPB - 1
                for p in range(2 * (j + 1)):
                    pair = (2 * p, 2 * p + 1)
                    st = ups.tile([PT, 2 * SB], F32, tag="u")
                    jb0, u0 = divmod(pair[0], TPB)
                    nc.tensor.matmul(st[:, 0:SB],
                                     lhsT=ktq[jb0][0:H, u0 * PT:(u0 + 1) * PT],
                                     rhs=qkt[j][0:H, :],
                                     start=True, stop=True)
                    jb1, u1 = divmod(pair[1], TPB)
                    nc.tensor.matmul(st[:, SB:2 * SB],
                                     lhsT=qkt[jb1][H:PT, u1 * PT:(u1 + 1) * PT],
                                     rhs=ktq[j][H:PT, :],
                                     start=True, stop=True)
                    ex = expp.tile([PT, 2 * SB], BF16, tag="exp")
                    nc.scalar.activation(ex, st, mybir.ActivationFunctionType.Exp,
                                         scale=scale)
                    for q, tt in enumerate(pair):
                        if tt >= j * TPB:  # diagonal tile: zero where t > s
                            nc.gpsimd.affine_select(
                                out=ex[:, q * SB:(q + 1) * SB],
                                in_=ex[:, q * SB:(q + 1) * SB],
                                compare_op=mybir.AluOpType.is_ge,
                                fill=0.0,
                                base=j * SB - tt * PT,
                                pattern=[[1, SB]],
                                channel_multiplier=-1,
                            )
                    for q, tt in enumerate(pair):
                        nc.tensor.matmul(ps_o, lhsT=vext[tt],
                                         rhs=ex[:, q * SB:(q + 1) * SB],
                                         start=(tt == 0), stop=(tt == last_tt))
                ot = otp.tile([H + 1, SB], F32, tag="ot")
                nc.vector.tensor_copy(ot, ps_o)

                ystage = yp.tile([PT, TPB, H], F32, tag="y")
                for u in range(TPB):
                    ps_f = ups.tile([PT, H + 1], F32, tag="u")
                    nc.tensor.matmul(ps_f, lhsT=ot[:, u * PT:(u + 1) * PT],
                                     rhs=wo_sb, start=True, stop=True)
                    rec = small.tile([PT, 1], F32, tag="rec")
                    nc.vector.reciprocal(rec, ps_f[:, H:H + 1])
                    nc.vector.tensor_scalar_mul(ystage[:, u, :], ps_f[:, 0:H], rec)
                nc.sync.dma_start(
                    out=y_d[j * SB:(j + 1) * SB, :].rearrange("(u p) g -> p u g", p=PT),
                    in_=ystage)

            finish_qk(0)
            finish_qk(1)
            nc.vector.tensor_scalar_add(vt2, vv, bv2_sb)
            finish_v(0)
            attn(0)
            attn(1)
            finish_qk(2)
            finish_qk(3)
            finish_v(1)
            attn(2)
            attn(3)
    nc.finalize()  # bacc compile: register alloc + wait splitting (TRN2: <=1 wait/inst)
    return nc


def get_nc():
    if "nc" not in _NC_CACHE:
        _NC_CACHE["nc"] = _build()
    return _NC_CACHE["nc"]


def make_in_maps(x, Wq, bq, Wk, bk, Wv, bv, Wo, bo):
    x = np.asarray(x, np.float32)
    # cst16 = [wqk(8 d-tiles x 128) | wv(8 x 64) | e12(128) | idn2(64)]
    wqk = np.concatenate([np.asarray(Wq).T, np.asarray(Wk).T], axis=1)  # [D, 128]
    wv = np.asarray(Wv).T  # [D, 64]
    cst16 = np.zeros((PT, 1728), np.float32)
    cst16[:, 0:NDT * PT] = wqk.reshape(NDT, PT, PT).transpose(1, 0, 2).reshape(PT, -1)
    cst16[:, NDT * PT:1536] = wv.reshape(NDT, PT, H).transpose(1, 0, 2).reshape(PT, -1)
    e12 = np.zeros((PT, PT), np.float32)  # swap: out rows = [in 64:128; in 0:64]
    e12[H + np.arange(H), np.arange(H)] = 1
    e12[np.arange(H), H + np.arange(H)] = 1
    cst16[:, 1536:1664] = e12
    cst16[0:H, 1664:1728] = np.eye(H)
    cst16[H:PT, 1664:1728] = np.eye(H)
    cst16 = cst16.astype(BF16NP)
    cst32 = np.zeros((PT, H + 3), np.float32)
    cst32[:H, :H] = np.asarray(Wo).T
    cst32[H, :H] = np.asarray(bo)
    cst32[H, H] = 1.0
    cst32[:, H + 1] = np.concatenate([np.asarray(bq), np.asarray(bk)])
    cst32[:, H + 2] = np.concatenate([np.asarray(bv), np.asarray(bv)])
    shared = {"cst16": cst16, "cst32": cst32}
    return [dict(shared, xt=np.ascontiguousarray(x[b].T).astype(BF16NP))
            for b in range(B)]


def kernel(**inputs):
    nc = get_nc()
    in_maps = make_in_maps(**inputs)
    res = run_bass_kernel_spmd(nc, in_maps, list(range(NCORES)))
    return np.stack([res.results[i]["y"] for i in range(NCORES)], axis=0)


# revision 11
# speedup vs baseline: 1.3563x; 1.3563x over previous
"""Trainium2 Bass kernel for a single-head causal attention block (v9).

Structure (informed by NTFF traces of v1-v7):
  - consts packed into 2 DMAs, issued first; then 8 big x^T d-tile DMAs
  - QK projections for ALL FOUR s-blocks accumulate d-tile-outer in four
    1-bank PSUM accumulators -> PE is dense while x^T streams in
  - V^T (col-packed, two s-blocks per pass) runs right after, from one
    2-bank score-pool tile; its second half plus the V transposes are
    interleaved as fillers between early attention pairs
  - attention: row-packed score matmul pairs -> one [128,1024] exp on ACT
    -> GPSIMD causal zero-fill on diagonal tiles -> PV accumulation with a
    carried ones-column (softmax denominator) -> fp32 output projection
    with [[Wo^T,0],[bo,1]] -> reciprocal * scale -> one DMA per s-block
  - PSUM: 4x1-bank accumulators (reused by swaps/PV-accum/fins) +
    2x2-bank score tiles = 8 banks
"""

import numpy as np
import ml_dtypes

import concourse.bacc as bacc
import concourse.mybir as mybir
from concourse.tile import TileContext
from concourse.bass_utils import run_bass_kernel_spmd

B, S, D, H = 8, 2048, 1024, 64
NCORES = 8
PT = 128           # partition tile
SB = 512           # s-block width
NSB = S // SB      # 4
NDT = D // PT      # 8 d-tiles
TPB = SB // PT     # 4 t-tiles per s-block
BF16 = mybir.dt.bfloat16
F32 = mybir.dt.float32
BF16NP = ml_dtypes.bfloat16

_NC_CACHE = {}


def _build():
    nc = bacc.Bacc()
    xt_d = nc.dram_tensor("xt", [D, S], BF16, kind="ExternalInput")
    cst16_d = nc.dram_tensor("cst16", [PT, 1728], BF16, kind="ExternalInput")
    cst32_d = nc.dram_tensor("cst32", [PT, H + 3], F32, kind="ExternalInput")
    y_d = nc.dram_tensor("y", [S, H], F32, kind="ExternalOutput")

    scale = 1.0 / float(np.sqrt(H))

    with TileContext(nc) as tc:
        with (
            tc.tile_pool(name="consts", bufs=1) as consts,
            tc.tile_pool(name="persist", bufs=1) as persist,
            tc.tile_pool(name="work", bufs=1) as work,
            tc.tile_pool(name="expp", bufs=3) as expp,
            tc.tile_pool(name="otp", bufs=2) as otp,
            tc.tile_pool(name="yp", bufs=2) as yp,
            tc.tile_pool(name="small", bufs=4) as small,
            tc.tile_pool(name="acc", bufs=4, space="PSUM") as accp,
            tc.tile_pool(name="stp", bufs=2, space="PSUM") as stp,
        ):
            # ---- tiny packed consts first (the first matmul needs wqk)
            cst16 = consts.tile([PT, 1728], BF16, tag="cst16")
            nc.sync.dma_start(out=cst16, in_=cst16_d[:])
            cst32 = consts.tile([PT, H + 3], F32, tag="cst32")
            nc.sync.dma_start(out=cst32, in_=cst32_d[:])
            xt_sb = persist.tile([PT, NDT, S], BF16, tag="xt")
            for k in range(NDT):
                nc.sync.dma_start(out=xt_sb[:, k, :],
                                  in_=xt_d[k * PT:(k + 1) * PT, :])
            wqk_sb = cst16[:, 0:NDT * PT].rearrange("p (k m) -> p k m", k=NDT)
            wv_sb = cst16[:, NDT * PT:NDT * PT + NDT * H].rearrange(
                "p (k m) -> p k m", k=NDT)
            e12_sb = cst16[:, 1536:1536 + PT]
            idn2_sb = cst16[:, 1664:1664 + H]
            wo_sb = cst32[0:H + 1, 0:H + 1]
            bqk_sb = cst32[:, H + 1:H + 2]
            bv2_sb = cst32[:, H + 2:H + 3]

            # warm the ACT exp table while DMA streams in
            warm_in = small.tile([PT, 1], F32, tag="warm_in")
            nc.vector.memset(warm_in, 0.0)
            warm_out = small.tile([PT, 1], F32, tag="warm_out")
            nc.scalar.activation(warm_out, warm_in, mybir.ActivationFunctionType.Exp)

            qkt = [None] * NSB   # [128, SB] bf16: rows 0:64 Q^T, 64:128 K^T
            ktq = [None] * NSB   # [128, SB] bf16: rows 0:64 K^T, 64:128 Q^T
            vext = [None] * (NSB * TPB)  # [128, H+1] bf16: [V | 1]

            # ---- projection d-loop: 8 matmuls per arriving d-tile
            # (QK for all four s-blocks + col-packed V^T for all four)
            a = []
            for _ in range(NSB):
                acc_t = accp.tile([PT, SB], F32, tag="acc")
                a.append(acc_t)
            vvt = stp.tile([PT, 2 * SB], F32, tag="st")
            for k in range(NDT):
                for j in range(NSB):
                    nc.tensor.matmul(a[j], lhsT=wqk_sb[:, k, :],
                                     rhs=xt_sb[:, k, j * SB:(j + 1) * SB],
                                     start=(k == 0), stop=(k == NDT - 1))
                for jp in range(2):
                    for h, j in ((0, 2 * jp), (H, 2 * jp + 1)):
                        nc.tensor.matmul(
                            vvt[h:h + H, jp * SB:(jp + 1) * SB],
                            lhsT=wv_sb[:, k, :],
                            rhs=xt_sb[:, k, j * SB:(j + 1) * SB],
                            start=(k == 0), stop=(k == NDT - 1),
                            skip_group_check=True)
            for j in range(NSB):
                qkt_j = persist.tile([PT, SB], BF16, tag=f"qkt{j}")
                nc.vector.tensor_scalar_add(qkt_j, a[j], bqk_sb)
                qkt[j] = qkt_j
                ps_sw = accp.tile([PT, SB], F32, tag="acc")
                nc.tensor.matmul(ps_sw, lhsT=e12_sb, rhs=qkt_j, start=True, stop=True)
                ktq_j = persist.tile([PT, SB], BF16, tag=f"ktq{j}")
                nc.vector.tensor_copy(ktq_j, ps_sw)
                ktq[j] = ktq_j

            # ---- V^T -> SBUF -> per-tile PE transposes with ones column
            vt2 = work.tile([PT, 2 * SB], BF16, tag="vt")
            nc.vector.tensor_scalar_add(vt2, vvt, bv2_sb)
            for j in range(NSB):
                rb = (j % 2) * H
                for u in range(TPB):
                    tt = j * TPB + u
                    col = (j // 2) * SB + u * PT
                    ps_vt = accp.tile([PT, H], BF16, tag="acc")
                    nc.tensor.transpose(ps_vt, vt2[rb:rb + H, col:col + PT],
                                        idn2_sb[rb:rb + H, :])
                    vx = persist.tile([PT, H + 1], BF16, tag=f"vext{tt}")
                    nc.vector.tensor_copy(vx[:, 0:H], ps_vt)
                    nc.vector.memset(vx[:, H:H + 1], 1.0)
                    vext[tt] = vx

            def attn(j):
                ps_o = accp.tile([H + 1, SB], F32, tag="acc")
                last_tt = (j + 1) * TPB - 1
                for p in range(2 * (j + 1)):
                    pair = (2 * p, 2 * p + 1)
                    st = stp.tile([PT, 2 * SB], F32, tag="st")
                    jb0, u0 = divmod(pair[0], TPB)
                    nc.tensor.matmul(st[:, 0:SB],
                                     lhsT=ktq[jb0][0:H, u0 * PT:(u0 + 1) * PT],
                                     rhs=qkt[j][0:H, :],
                                     start=True, stop=True)
                    jb1, u1 = divmod(pair[1], TPB)
                    nc.tensor.matmul(st[:, SB:2 * SB],
                                     lhsT=qkt[jb1][H:PT, u1 * PT:(u1 + 1) * PT],
                                     rhs=ktq[j][H:PT, :],
                                     start=True, stop=True)
                    ex = expp.tile([PT, 2 * SB], BF16, tag="exp")
                    nc.scalar.activation(ex, st, mybir.ActivationFunctionType.Exp,
                                         scale=scale)
                    for q, tt in enumerate(pair):
                        if tt >= j * TPB:  # diagonal tile: zero where t > s
                            nc.gpsimd.affine_select(
                                out=ex[:, q * SB:(q + 1) * SB],
                                in_=ex[:, q * SB:(q + 1) * SB],
                                compare_op=mybir.AluOpType.is_ge,
                                fill=0.0,
                                base=j * SB - tt * PT,
                                pattern=[[1, SB]],
                                channel_multiplier=-1,
                            )
                    for q, tt in enumerate(pair):
                        nc.tensor.matmul(ps_o, lhsT=vext[tt],
                                         rhs=ex[:, q * SB:(q + 1) * SB],
                                         start=(tt == 0), stop=(tt == last_tt))
                ot = otp.tile([H + 1, SB], F32, tag="ot")
                nc.vector.tensor_copy(ot, ps_o)

                ystage = yp.tile([PT, TPB, H], F32, tag="y")
                for u in range(TPB):
                    ps_f = accp.tile([PT, H + 1], F32, tag="acc")
                    nc.tensor.matmul(ps_f, lhsT=ot[:, u * PT:(u + 1) * PT],
                                     rhs=wo_sb, start=True, stop=True)
                    rec = small.tile([PT, 1], F32, tag="rec")
                    nc.vector.reciprocal(rec, ps_f[:, H:H + 1])
                    nc.vector.tensor_scalar_mul(ystage[:, u, :], ps_f[:, 0:H], rec)
                nc.sync.dma_start(
                    out=y_d[j * SB:(j + 1) * SB, :].rearrange("(u p) g -> p u g", p=PT),
                    in_=ystage)

            for j in range(NSB):
                attn(j)
    nc.finalize()  # bacc compile: register alloc + wait splitting (TRN2: <=1 wait/inst)
    return nc


def get_nc():
    if "nc" not in _NC_CACHE:
        _NC_CACHE["nc"] = _build()
    return _NC_CACHE["nc"]


def make_in_maps(x, Wq, bq, Wk, bk, Wv, bv, Wo, bo):
    x = np.asarray(x, np.float32)
    # cst16 = [wqk(8 d-tiles x 128) | wv(8 x 64) | e12(128) | idn2(64)]
    wqk = np.concatenate([np.asarray(Wq).T, np.asarray(Wk).T], axis=1)  # [D, 128]
    wv = np.asarray(Wv).T  # [D, 64]
    cst16 = np.zeros((PT, 1728), np.float32)
    cst16[:, 0:NDT * PT] = wqk.reshape(NDT, PT, PT).transpose(1, 0, 2).reshape(PT, -1)
    cst16[:, NDT * PT:1536] = wv.reshape(NDT, PT, H).transpose(1, 0, 2).reshape(PT, -1)
    e12 = np.zeros((PT, PT), np.float32)  # swap: out rows = [in 64:128; in 0:64]
    e12[H + np.arange(H), np.arange(H)] = 1
    e12[np.arange(H), H + np.arange(H)] = 1
    cst16[:, 1536:1664] = e12
    cst16[0:H, 1664:1728] = np.eye(H)
    cst16[H:PT, 1664:1728] = np.eye(H)
    cst16 = cst16.astype(BF16NP)
    cst32 = np.zeros((PT, H + 3), np.float32)
    cst32[:H, :H] = np.asarray(Wo).T
    cst32[H, :H] = np.asarray(bo)
    cst32[H, H] = 1.0
    cst32[:, H + 1] = np.concatenate([np.asarray(bq), np.asarray(bk)])
    cst32[:, H + 2] = np.concatenate([np.asarray(bv), np.asarray(bv)])
    shared = {"cst16": cst16, "cst32": cst32}
    return [dict(shared, xt=np.ascontiguousarray(x[b].T).astype(BF16NP))
            for b in range(B)]


def kernel(**inputs):
    nc = get_nc()
    in_maps = make_in_maps(**inputs)
    res = run_bass_kernel_spmd(nc, in_maps, list(range(NCORES)))
    return np.stack([res.results[i]["y"] for i in range(NCORES)], axis=0)
